# revision 29
# baseline (speedup 1.0000x reference)
"""Transformer-XL multi-head attention on 8 trn2 NeuronCores.

Sharding: tensor-parallel over heads (2 heads/core x 16 heads), all batches on
every core. Host sums the per-core partial output projections (f16 partials).

Per-core pipeline (fp8 e4m3 DoubleRow matmuls for all score-space work):
  1. Projections from host-prepped fp8 operands (weights pre-scaled x16, the
     1/16 folded back in the PSUM->SBUF conversion copies):
       k/q/p: fp8 DoubleRow (real k-tile pairs along the free dim).
       v: 3-term fp8 DoubleRow (x_hi@w_hi + x_hi@w_lo + x_lo@w_hi) so the
          value path keeps ~1e-3 accuracy; the x16 weight scale is folded
          into W_out on the host instead of a per-copy rescale.
  2. Position scores PD[i,t] = 2*(q_i+v).p_t via stride-0 DoubleRow (the "2"
     k-tile pair aliases the same data; every score-space matmul carries the
     same x2, cancelled by exp(scale/2)). PD is written to a per-(b,h) DRAM
     buffer G3 with a skewed AP: addr = B0 + i*(GP+1) + (t-896), so that the
     TXL rel-shift becomes plain row reads: G3[addr B0 + i*GP + j + 127] =
     pos[j,i]. Masked cells (t>=2048) are never written; a one-time guard
     fill of -448 covers exactly the readable masked range.
  3. Readback is a plain full-bandwidth DMA (512B+ rows) into [i,j]-oriented
     fp8 tiles; a DoubleRow identity matmul transposes each [128i x 128j]
     tile directly into the content-score PSUM accumulator (the transpose IS
     the content+position add).
  4. Content scores fp8 DoubleRow in [j,i] layout; one exp per (b,ib,jt)
     over both heads' PSUM banks -> bf16 attention weights.
  5. PV bf16 with a ones-column (row 64 = softmax denominator), normalize via
     reciprocal + PE broadcast, output projection bf16, f16 partial out.

Emission is manually weaved: attention tiles of batch b interleave with
projection/PD work of batch b+1 and the output projection of b-1, so every
engine's in-order stream mixes foreground and background work.
"""

import numpy as np
import ml_dtypes

import concourse.bass as bass
from concourse import bacc
import concourse.mybir as mybir
import concourse.tile as tile
from concourse.bass_utils import run_bass_kernel_spmd

BF16 = mybir.dt.bfloat16
F32 = mybir.dt.float32
F16 = mybir.dt.float16
FP8 = mybir.dt.float8e4
AF = mybir.ActivationFunctionType
DR = mybir.MatmulPerfMode.DoubleRow

CS, PS, BS, DIN, H, D = 1024, 1024, 4, 1024, 16, 64
KS = CS + PS                  # 2048 keys
NCORES = 8
HPC = H // NCORES             # 2 heads per core
TOK = KS * BS                 # kv tokens, b-major
QTOK = CS * BS
SCALE = 1.0 / D ** 0.5
NKP = DIN // 256              # 4 contraction k-tile pairs (DoubleRow)
GP1 = 3072                    # G3 write pitch
GP = GP1 - 1                  # G3 read pitch (the skew)
B0 = 896                      # G3 base offset so min write addr is 0
GUARD = -240.0                # max-negative finite e4m3 (has inf!); exp ~ 0
WSCALE = 16.0                 # host pre-scale on all projection weights

_CACHED = {}


def _s0(ap, shape):
    """Insert a stride-0 k-tile-pair dim after the partition dim."""
    return ap.unsqueeze(1).broadcast_to(shape)


def _jts(ib):
    """Included j-tiles (128 wide) for i-half ib (512 wide)."""
    return list(range(12 if ib == 0 else 16))


def _weave(fg, bg):
    """Emit fg and bg step lists interleaved proportionally."""
    if not fg:
        for s in bg:
            s()
        return
    r = len(bg) / len(fg)
    acc = 0.0
    bi = 0
    for s in fg:
        s()
        acc += r
        while bi < len(bg) and bi < int(acc):
            bg[bi]()
            bi += 1
    while bi < len(bg):
        bg[bi]()
        bi += 1


def build_nc():
    nc = bacc.Bacc()
    x8 = nc.declare_dram_parameter("x8", [128, NKP, 2, TOK], FP8, isOutput=False)
    xr8 = nc.declare_dram_parameter("xr8", [128, NKP, 2, TOK], FP8, isOutput=False)
    pos8 = nc.declare_dram_parameter("pos8", [128, NKP, 2, KS], FP8, isOutput=False)
    wk8 = nc.declare_dram_parameter("wk8", [128, NKP, 2, 128], FP8, isOutput=False)
    wq8 = nc.declare_dram_parameter("wq8", [128, NKP, 2, 128], FP8, isOutput=False)
    wp8 = nc.declare_dram_parameter("wp8", [128, NKP, 2, 128], FP8, isOutput=False)
    wv8 = nc.declare_dram_parameter("wv8", [128, NKP, 2, 128], FP8, isOutput=False)
    wvr8 = nc.declare_dram_parameter("wvr8", [128, NKP, 2, 128], FP8, isOutput=False)
    wout = nc.declare_dram_parameter("wout", [128, DIN], BF16, isOutput=False)
    ucol = nc.declare_dram_parameter("ucol", [128, 1], F32, isOutput=False)
    vcol = nc.declare_dram_parameter("vcol", [128, 1], F32, isOutput=False)
    ident = nc.declare_dram_parameter("ident", [128, 2, 256], FP8, isOutput=False)
    gneg = nc.declare_dram_parameter("gneg", [128, 512], FP8, isOutput=False)
    out = nc.declare_dram_parameter("out", [CS, BS, DIN], F16, isOutput=True)

    g3 = [[nc.dram_tensor(f"g3_{b}_{h}", [1024 * GP1], FP8)
           for h in range(HPC)] for b in range(BS)]

    with tile.TileContext(nc) as tc:
        _body(nc, tc, x8, xr8, pos8, wk8, wq8, wp8, wv8, wvr8, wout,
              ucol, vcol, ident, gneg, out, g3)
    nc.compile()
    return nc


def _body(nc, tc, x8, xr8, pos8, wk8, wq8, wp8, wv8, wvr8, wout,
          ucol, vcol, ident, gneg, out, g3):
    import contextlib

    ctx = contextlib.ExitStack()
    with ctx:
        res = ctx.enter_context(tc.tile_pool(name="res", bufs=1))
        xtp = ctx.enter_context(tc.tile_pool(name="xtp", bufs=4))
        pdb = ctx.enter_context(tc.tile_pool(name="pdb", bufs=6))
        pob = ctx.enter_context(tc.tile_pool(name="pob", bufs=2))
        att = ctx.enter_context(tc.tile_pool(name="att", bufs=8))
        wrk = ctx.enter_context(tc.tile_pool(name="wrk", bufs=4))
        ost = ctx.enter_context(tc.tile_pool(name="ost", bufs=1))
        ps_mm = ctx.enter_context(tc.tile_pool(name="ps_mm", bufs=2, space="PSUM"))
        ps_cn = ctx.enter_context(tc.tile_pool(name="ps_cn", bufs=2, space="PSUM"))
        ps_pv = ctx.enter_context(tc.tile_pool(name="ps_pv", bufs=1, space="PSUM"))

        # ---- resident tiles
        kT8 = res.tile([128, TOK], FP8)            # [(h,d), b*KS+j]
        qu8 = res.tile([128, QTOK], FP8)           # [(h,d), b*CS+i] (q+u)
        qv8 = res.tile([128, QTOK], FP8)           # (q+v)
        pT8 = res.tile([128, KS], FP8)
        vaug = res.tile([128, BS * HPC * 16, 65], BF16)   # V + ones col
        outT = res.tile([128, BS, CS], BF16)
        w_k = res.tile([128, NKP, 2, 128], FP8)
        w_q = res.tile([128, NKP, 2, 128], FP8)
        w_p = res.tile([128, NKP, 2, 128], FP8)
        w_v = res.tile([128, NKP, 2, 128], FP8)
        w_vr = res.tile([128, NKP, 2, 128], FP8)
        w_o = res.tile([128, DIN], BF16)
        u_sb = res.tile([128, 1], F32)
        v_sb = res.tile([128, 1], F32)
        id8 = res.tile([128, 2, 256], FP8)
        gn = res.tile([128, 512], FP8)
        ones64 = res.tile([1, 64], F32)
        pp8 = res.tile([128, NKP, 2, KS], FP8)     # pos_embs operand

        for t, src in ((w_k, wk8), (w_q, wq8), (w_p, wp8), (w_v, wv8),
                       (w_vr, wvr8)):
            nc.sync.dma_start(out=t[:], in_=src[:, :, :, :])
        nc.sync.dma_start(out=w_o[:], in_=wout[:, :])
        nc.sync.dma_start(out=u_sb[:], in_=ucol[:, :])
        nc.sync.dma_start(out=v_sb[:], in_=vcol[:, :])
        nc.sync.dma_start(out=id8[:], in_=ident[:, :, :])
        nc.sync.dma_start(out=gn[:], in_=gneg[:, :])
        nc.vector.memset(ones64[:], 1.0)
        nc.vector.memset(vaug[:, :, 64:65], 1.0)

        def guard_fill(half):
            # G3 guard: per (b,h) rows half*512..+512, rel cols [1152, 1664)
            for b in range(BS):
                for h in range(HPC):
                    dst = bass.AP(tensor=g3[b][h],
                                  offset=B0 + half * 512 * GP1 + 1152,
                                  ap=[[GP1, 512], [1, 512]])
                    nc.scalar.dma_start(
                        out=dst,
                        in_=gn[:].unsqueeze(1).broadcast_to([128, 4, 512]))

        def p_proj():
            # pT8[:, t] (2 heads stacked on partitions)
            nc.sync.dma_start(out=pp8[:], in_=pos8[:, :, :, :])
            for tt in range(KS // 512):
                acc = ps_mm.tile([128, 512], F32, tag="mm")
                for kp in range(NKP):
                    nc.tensor.matmul(
                        acc[:], w_p[:, kp, :, :],
                        pp8[:, kp, :, tt * 512:(tt + 1) * 512],
                        start=(kp == 0), stop=(kp == NKP - 1), perf_mode=DR)
                nc.scalar.mul(pT8[:, tt * 512:(tt + 1) * 512], acc[:], 1.0 / WSCALE)

        # ---------------- step builders ----------------

        def proj_steps(b, tts=(0, 1, 2, 3)):
            """k/v for the 2048 tokens of b; q for its last 1024."""
            steps = []
            xts = {}

            def kq_step(tt4):
                tok0 = b * KS + tt4 * 512
                xt = xtp.tile([128, NKP, 2, 512], FP8, tag="x8",
                              name=f"xt{b}_{tt4}")
                xr = xtp.tile([128, NKP, 2, 512], FP8, tag="xr8",
                              name=f"xr{b}_{tt4}")
                nc.sync.dma_start(out=xt[:], in_=x8[:, :, :, tok0:tok0 + 512])
                nc.sync.dma_start(out=xr[:], in_=xr8[:, :, :, tok0:tok0 + 512])
                xts[tt4] = (xt, xr)
                if tt4 >= 2:
                    q0 = b * CS + (tt4 - 2) * 512
                    accq = ps_mm.tile([128, 512], F32, tag="mm", name="accq")
                    for kp in range(NKP):
                        nc.tensor.matmul(
                            accq[:], w_q[:, kp, :, :], xt[:, kp, :, :],
                            start=(kp == 0), stop=(kp == NKP - 1), perf_mode=DR)
                    nc.vector.tensor_scalar(
                        qv8[:, q0:q0 + 512], accq[:], 1.0 / WSCALE,
                        v_sb[:], mybir.AluOpType.mult, mybir.AluOpType.add)
                    nc.scalar.activation(
                        qu8[:, q0:q0 + 512], accq[:], AF.Identity,
                        bias=u_sb[:], scale=1.0 / WSCALE)
                acc = ps_mm.tile([128, 512], F32, tag="mm", name="acck")
                for kp in range(NKP):
                    nc.tensor.matmul(
                        acc[:], w_k[:, kp, :, :], xt[:, kp, :, :],
                        start=(kp == 0), stop=(kp == NKP - 1), perf_mode=DR)
                nc.vector.tensor_scalar_mul(kT8[:, tok0:tok0 + 256],
                                            acc[:, 0:256], 1.0 / WSCALE)
                nc.scalar.mul(kT8[:, tok0 + 256:tok0 + 512],
                              acc[:, 256:512], 1.0 / WSCALE)

            def v_step(tt4, s2):
                xt, xr = xts[tt4]
                for sub in (2 * s2, 2 * s2 + 1):
                    accv = ps_mm.tile([128, 128], F32, tag="mm", name="accv")
                    n = 0
                    for xa, wa in ((xt, w_v), (xt, w_vr), (xr, w_v)):
                        for kp in range(NKP):
                            nc.tensor.matmul(
                                accv[:], xa[:, kp, :, sub * 128:sub * 128 + 128],
                                wa[:, kp, :, :],
                                start=(n == 0), stop=(n == 11), perf_mode=DR)
                            n += 1
                    jt = tt4 * 4 + sub
                    slot0 = b * HPC * 16 + jt
                    nc.vector.tensor_copy(
                        vaug[:, slot0:slot0 + 17:16, 0:64], accv[:])

            for tt4 in tts:
                steps.append(lambda tt4=tt4: kq_step(tt4))
                for s2 in range(2):
                    steps.append(lambda tt4=tt4, s2=s2: v_step(tt4, s2))
            return steps

        pd_piece_ctr = [0]

        def pd_read_steps(b, posb_out, a_lo=0, a_hi=8):
            """PD strips for b (interleaved h), plus posb readbacks as soon
            as each i-half's strips are written."""
            steps = []

            def strip(a, h):
                tstart = 896 - 128 * a
                i0 = b * CS + a * 128
                hp = slice(h * 64, h * 64 + 64)
                W = 2048 - tstart
                bounds = [tstart]
                nb = (tstart + 511) // 512 * 512
                while nb <= 2048:
                    if nb > bounds[-1]:
                        bounds.append(nb)
                    nb += 512
                stg = pdb.tile([128, 2048], FP8, tag="pdstg", name="stg")
                for lo, hi in zip(bounds[:-1], bounds[1:]):
                    w = hi - lo
                    accp = ps_mm.tile([128, 512], F32, tag="mm", name="accp")
                    nc.tensor.matmul(
                        accp[:, 0:w],
                        _s0(qv8[hp, i0:i0 + 128], [64, 2, 128]),
                        _s0(pT8[hp, lo:hi], [64, 2, w]),
                        start=True, stop=True, perf_mode=DR)
                    pd_piece_ctr[0] += 1
                    if pd_piece_ctr[0] % 5 < 1:
                        nc.scalar.mul(stg[:, lo - tstart:hi - tstart],
                                      accp[:, 0:w], 1.0)
                    else:
                        nc.vector.tensor_copy(stg[:, lo - tstart:hi - tstart],
                                              accp[:, 0:w])
                dst = bass.AP(tensor=g3[b][h], offset=B0 + 128 * a * GP,
                              ap=[[GP1, 128], [1, W]])
                nc.sync.dma_start(out=dst, in_=stg[:, 0:W])

            def read(half):
                Wr = 1536 if half == 0 else 2048
                for h in range(HPC):
                    po = pob.tile([128, 4, Wr], FP8, tag=f"po{half}",
                                  name=f"po_{h}_{half}")
                    src = bass.AP(tensor=g3[b][h],
                                  offset=B0 + 127 + (half * 4 * 128) * GP,
                                  ap=[[128 * GP, 4], [GP, 128], [1, Wr]])
                    nc.sync.dma_start(out=po[:], in_=src)
                    posb_out[(h, half)] = po

            for a in range(a_lo, a_hi):
                for h in range(HPC):
                    steps.append(lambda a=a, h=h: strip(a, h))
                if a == 3:
                    steps.append(lambda: read(0))
                if a == 7:
                    steps.append(lambda: read(1))
            return steps

        def attn_steps(b, posb):
            steps = []
            pvs = {}

            def tile_step(ib, n, jt, last):
                i_sl = slice(b * CS + ib * 512, b * CS + ib * 512 + 512)
                j_sl = slice(b * KS + jt * 128, b * KS + jt * 128 + 128)
                if n == 0:
                    pvs[ib] = [ps_pv.tile([65, 512], F32, tag=f"pv{h}",
                                          name=f"pvp{h}") for h in range(HPC)]
                pvp = pvs[ib]
                icl = max(0, jt * 128 - PS - ib * 512)
                i_cl = slice(b * CS + ib * 512 + icl, b * CS + ib * 512 + 512)
                iw = 512 - icl
                cn = ps_cn.tile([128, 1024], F32, tag="cn")
                for h in range(HPC):
                    hp = slice(h * 64, h * 64 + 64)
                    nc.tensor.matmul(
                        cn[:, h * 512 + icl:h * 512 + 512],
                        _s0(kT8[hp, j_sl], [64, 2, 128]),
                        _s0(qu8[hp, i_cl], [64, 2, iw]),
                        start=True, stop=False, perf_mode=DR)
                    po = posb[(h, ib)]
                    for cp in range(icl // 256, 2):
                        lo = max(0, icl - cp * 256)
                        nc.tensor.matmul(
                            cn[:, h * 512 + cp * 256 + lo:h * 512 + cp * 256 + 256],
                            po[:, cp * 2:cp * 2 + 2, jt * 128:jt * 128 + 128],
                            id8[:, :, lo:256],
                            start=False, stop=(cp == 1), perf_mode=DR,
                            skip_group_check=True)
                atn = att.tile([128, 1024], BF16, tag="atn")
                if icl:
                    nc.scalar.activation(atn[:, icl:512], cn[:, icl:512],
                                         AF.Exp, scale=SCALE / 2)
                    nc.scalar.activation(atn[:, 512 + icl:1024],
                                         cn[:, 512 + icl:1024],
                                         AF.Exp, scale=SCALE / 2)
                else:
                    nc.scalar.activation(atn[:], cn[:], AF.Exp, scale=SCALE / 2)
                for h in range(HPC):
                    slot = b * HPC * 16 + h * 16 + jt
                    nc.tensor.matmul(
                        pvp[h][:, icl:512], vaug[:, slot, :],
                        atn[:, h * 512 + icl:h * 512 + 512],
                        start=(n == 0), stop=last)

            def norm(ib, h):
                pvp = pvs[ib]
                rec = wrk.tile([1, 512], F32, tag="rec")
                nc.vector.reciprocal(rec[:], pvp[h][64:65, :])
                rbp = ps_mm.tile([64, 512], F32, tag="mm")
                nc.tensor.matmul(rbp[:], ones64[:], rec[:], start=True, stop=True)
                rbs = wrk.tile([64, 512], F32, tag="rbs")
                nc.scalar.mul(rbs[:], rbp[:], 1.0)
                nc.vector.tensor_mul(
                    outT[h * 64:h * 64 + 64, b, ib * 512:ib * 512 + 512],
                    pvp[h][0:64, :], rbs[:])

            for ib in range(2):
                jts = _jts(ib)
                for n, jt in enumerate(jts):
                    steps.append(lambda ib=ib, n=n, jt=jt,
                                 last=(n == len(jts) - 1): tile_step(ib, n, jt, last))
                for h in range(HPC):
                    steps.append(lambda ib=ib, h=h: norm(ib, h))
            return steps

        osb_of = {}

        def outproj_steps(b, ib):
            steps = []

            def mmcopy(it, dh):
                if b not in osb_of:
                    osb_of[b] = ost.tile([128, 8, DIN], F16, tag="osb",
                                         name=f"osb{b}")
                osb = osb_of[b]
                accd = ps_mm.tile([128, 512], F32, tag="mm", name="accd")
                nc.tensor.matmul(
                    accd[:], outT[:, b, it * 128:it * 128 + 128],
                    w_o[:, dh * 512:dh * 512 + 512], start=True, stop=True)
                nc.vector.tensor_copy(osb[:, it, dh * 512:dh * 512 + 256],
                                      accd[:, 0:256])
                nc.scalar.mul(osb[:, it, dh * 512 + 256:dh * 512 + 512],
                              accd[:, 256:512], 1.0)

            for it in range(ib * 4, ib * 4 + 4):
                for dh in range(2):
                    steps.append(lambda it=it, dh=dh: mmcopy(it, dh))

            if ib == 1:
                def wr():
                    nc.sync.dma_start(
                        out=out.ap()[:, b, :].rearrange("(a p) n -> p a n", p=128),
                        in_=osb_of[b][:])
                steps.append(wr)
            return steps

        # ---------------- phase weave ----------------
        posb = [dict() for _ in range(BS)]
        # Startup: q-bearing token tile first so PD can begin; p-proj and
        # guard fills overlap; first i-half of PD(0) + its readback ASAP.
        for s in proj_steps(0, tts=(2,)):
            s()
        p_proj()
        guard_fill(0)
        _weave(proj_steps(0, tts=(0, 3, 1)),
               pd_read_steps(0, posb[0], 0, 4) + [lambda: guard_fill(1)])
        for b in range(BS):
            bg = []
            if b == 0:
                bg += pd_read_steps(0, posb[0], 4, 8)
            if b > 0:
                bg += outproj_steps(b - 1, 1)
            if b + 1 < BS:
                bg += proj_steps(b + 1) + pd_read_steps(b + 1, posb[b + 1])
            fg = attn_steps(b, posb[b])
            # this b's first-half outproj joins the tail of its own phase
            _weave(fg, bg)
            for s in outproj_steps(b, 0):
                s()
        for s in outproj_steps(BS - 1, 1):
            s()


def _host_prep(input_, pos_embs, memory, u, v, W_kv, W_q, W_p, W_out):
    f8 = ml_dtypes.float8_e4m3
    f32 = np.float32

    xmem = np.concatenate([memory, input_], axis=0)          # (KS, BS, DIN)
    XT = np.ascontiguousarray(xmem.transpose(2, 1, 0).reshape(DIN, TOK))
    X8 = XT.astype(f8)
    XR8 = (XT - X8.astype(f32)).astype(f8)

    def dsplit(m, cols):
        # [DIN, cols] -> [128, NKP, 2, cols] with din = (kp*2 + t)*128 + p
        return np.ascontiguousarray(
            m.reshape(NKP, 2, 128, cols).transpose(2, 0, 1, 3))

    X8 = dsplit(X8.astype(f32), TOK).astype(f8)
    XR8 = dsplit(XR8.astype(f32), TOK).astype(f8)
    P8 = dsplit(np.ascontiguousarray(pos_embs.reshape(KS, DIN).T), KS).astype(f8)

    id256 = np.zeros((128, 2, 256), dtype=f32)
    for k in range(128):
        id256[k, 0, k] = 1.0
        id256[k, 1, 128 + k] = 1.0
    ident = id256.astype(f8)
    gneg = np.full((128, 512), GUARD, dtype=f32).astype(f8)

    maps = []
    for c in range(NCORES):
        h0 = c * HPC
        sl = slice(h0 * D, h0 * D + HPC * D)
        wv16 = WSCALE * W_kv[:, H * D + h0 * D:H * D + h0 * D + HPC * D]
        wv8 = wv16.astype(f8)
        wvr8 = (wv16 - wv8.astype(f32)).astype(f8)
        maps.append({
            "x8": X8,
            "xr8": XR8,
            "pos8": P8,
            "wk8": dsplit(WSCALE * W_kv[:, sl], 128).astype(f8),
            "wq8": dsplit(WSCALE * W_q[:, sl], 128).astype(f8),
            "wp8": dsplit(WSCALE * W_p[:, sl], 128).astype(f8),
            "wv8": dsplit(wv8.astype(f32), 128).astype(f8),
            "wvr8": dsplit(wvr8.astype(f32), 128).astype(f8),
            "wout": (W_out[sl, :] / WSCALE).astype(ml_dtypes.bfloat16),
            "ucol": u[h0:h0 + HPC].reshape(HPC * D, 1).astype(f32),
            "vcol": v[h0:h0 + HPC].reshape(HPC * D, 1).astype(f32),
            "ident": ident,
            "gneg": gneg,
        })
    return maps


def kernel(input_, pos_embs, memory, u, v, W_kv, W_q, W_p, W_out, mask,
           _trace=False):
    if "nc" not in _CACHED:
        _CACHED["nc"] = build_nc()
    nc = _CACHED["nc"]
    args = [np.asarray(a, dtype=np.float32) for a in
            (input_, pos_embs, memory, u, v, W_kv, W_q, W_p, W_out)]
    in_maps = _host_prep(*args)
    res = run_bass_kernel_spmd(nc, in_maps, list(range(NCORES)), trace=_trace)
    total = np.zeros((CS, BS, DIN), np.float32)
    for r in res.results:
        total += r["out"].astype(np.float32)
    if _trace:
        _CACHED["last_results"] = res
    return total
